# revision 13
# baseline (speedup 1.0000x reference)
"""Block-quantize kernel for Trainium2 (8 NeuronCores, data-parallel).

Reference semantics (fp32, wl=8, ebit=8):
    m  = max(max|x|, 1e-10)                      # global over all elements
    e  = clip(floor(log2(m)), -128, 127)
    y  = clip(round_half_even(x * 2^(6-e)), -128, 127) * 2^(e-6)

Single-pass implementation:
  - x (16, 2048, 4096) f32 is sharded on the batch dim: 2 batches per core
    (64 MiB), treated as a flat per-core vector so every [128, TILE_F] tile
    is one contiguous DMA.
  - Each tile is quantized with the exponent of ITS OWN abs-max, streaming:
    load (SP queue) -> absmax reduce (DVE) -> partition all-reduce (Pool) ->
    derive the two power-of-two scales with exact int32 bit arithmetic
    (DVE, tiny [P,1] ops) -> i8 = sat_int8(x*s1) (DVE tensor_scalar with
    int8 output: the f32->int8 convert is round-to-nearest-even +
    saturating, verified on HW, so round AND clip fuse into the cast) ->
    y = i8 * s2 (DVE, bf16 output) -> store (ACT queue).
  - The output is written as bf16: every representable result i*2^(e-6),
    i in [-128,127], is EXACTLY representable in bf16 (8 significand
    bits), so this is a lossless encoding that halves the store traffic;
    the host widens back to f32 (exact).  HBM traffic: 64 MiB read +
    32 MiB write = 96 MiB/core (vs 174 MiB for the two-pass baseline).
  - Loads and stores are issued from different DGE queues (SP vs ACT) --
    sharing one queue serializes them and costs ~10% bandwidth.
  - The per-tile abs-maxes are a second (tiny) kernel output.  On the host,
    the global exponent e is derived from them; every tile whose local
    exponent equals e (and is >= -120, keeping all scales/outputs normal)
    is bit-exact, because the quantization grid only depends on the
    exponent's octave.  Any other tile (probability ~2^-47 for the
    gaussian input: a 512K-element tile's max falling an octave below the
    global max) is re-quantized exactly on the host in numpy.  The result
    is therefore exact for EVERY input, and the device does one pass for
    the overwhelmingly-likely case.
"""
import sys

if "/opt/trn_rl_repo" not in sys.path:
    sys.path.insert(0, "/opt/trn_rl_repo")

import numpy as np

N_CORES = 8
B, S, D = 16, 2048, 4096          # full input shape
PB = B // N_CORES                  # batches per core
P = 128                            # SBUF partitions
NELEM = PB * S * D                 # per-core elements (16.8M, 64 MiB)
TILE_F = 4096                      # tile free dim -> [128, 4096] = 2 MiB
BUFS = 6                           # f32 streaming-pool slots
BUFS2 = 5                          # int8/bf16 tile-pool slots
N_T = NELEM // (P * TILE_F)        # tiles per core
C_MAGIC = 12582912.0               # 1.5 * 2^23, round-to-nearest-even magic

_CACHE = {}


def _build(reps: int = 1, tile_f: int = TILE_F, bufs: int = BUFS,
           bufs2: int = BUFS2, quant: str = "int8", out_dtype: str = "bf16",
           load_engine: str = "sync", store_engine: str = "gpsimd,scalar",
           dequant_engine: str = "scalar", alt_quant: bool = True,
           tile_order: str = "split4"):
    import concourse.mybir as mybir
    from concourse import bacc, bass_isa, tile

    DT = mybir.dt.float32
    DI = mybir.dt.int32
    DO = mybir.dt.float32 if out_dtype == "f32" else mybir.dt.bfloat16
    A = mybir.AluOpType

    ch = P * tile_f                # elements per tile
    n_t = NELEM // ch              # tiles per pass
    assert n_t * ch == NELEM

    nc = bacc.Bacc("TRN2", target_bir_lowering=False, debug=False,
                   num_devices=N_CORES)
    x = nc.dram_tensor("x", [NELEM], DT, kind="ExternalInput")
    y = nc.dram_tensor("y", [NELEM], DO, kind="ExternalOutput")
    st = nc.dram_tensor("st", [reps * n_t], DT, kind="ExternalOutput")

    def blk(dram, i):
        return dram[i * ch:(i + 1) * ch].rearrange("(p f) -> p f", f=tile_f)

    load_engs = load_engine.split(",")
    store_engs = store_engine.split(",")

    with tile.TileContext(nc) as tc:
        with tc.tile_pool(name="data", bufs=bufs) as data, \
             tc.tile_pool(name="sm", bufs=bufs) as sm, \
             tc.tile_pool(name="big", bufs=bufs2) as big, \
             tc.tile_pool(name="persist", bufs=reps) as persist:
          for rep in range(reps):
            # per-tile abs-maxes, kept resident; DMA'd out once at the end
            stats = persist.tile([P, n_t], DT, tag="stats")
            # splitN: walk N contiguous regions of the shard interleaved --
            # N concurrent sequential DRAM streams measure much faster than
            # one (split4 ~25% over split2 over seq: HBM bank parallelism)
            if tile_order == "split2":
                h = n_t // 2
                order = [(j // 2) + h * (j % 2) for j in range(n_t)]
            elif tile_order == "split4":
                h = n_t // 4
                order = [(j // 4) + h * (j % 4) for j in range(n_t)]
            elif tile_order == "split8":
                h = n_t // 8
                order = [(j // 8) + h * (j % 8) for j in range(n_t)]
            elif tile_order == "split16":
                h = n_t // 16
                order = [(j // 16) + h * (j % 16) for j in range(n_t)]
            else:
                order = list(range(n_t))
            for j, i in enumerate(order):
                t = data.tile([P, tile_f], DT, tag="blk")
                getattr(nc, load_engs[j % len(load_engs)]).dma_start(
                    out=t[:], in_=blk(x, i))
                # ---- per-tile abs-max -> replicated scalar (exact) ----
                red = sm.tile([P, 1], DT, tag="red")
                nc.vector.tensor_reduce(out=red[:], in_=t[:],
                                        axis=mybir.AxisListType.X,
                                        op=A.max, apply_absolute_value=True)
                a = stats[:, i:i + 1]
                nc.gpsimd.partition_all_reduce(a, red[:], channels=P,
                                               reduce_op=bass_isa.ReduceOp.max)
                # ---- scales via exact bit arithmetic ----
                #   p   = bits(m) & 0x7F800000           # bits of 2^e
                #   s2i = p - (6<<23)                    # bits of 2^(e-6)
                #   s1i = (254<<23) - s2i                # bits of 2^(6-e)
                p_i = sm.tile([P, 1], DI, tag="p_i")
                nc.vector.tensor_scalar(out=p_i[:], in0=a.bitcast(DI),
                                        scalar1=0x7F800000, scalar2=None,
                                        op0=A.bitwise_and)
                s2i = sm.tile([P, 1], DI, tag="s2i")
                nc.vector.tensor_scalar(out=s2i[:], in0=p_i[:],
                                        scalar1=6 << 23, scalar2=None,
                                        op0=A.subtract)
                s2 = s2i[:].bitcast(DT)
                s1i = sm.tile([P, 1], DI, tag="s1i")
                nc.vector.tensor_scalar(out=s1i[:], in0=s2i[:],
                                        scalar1=254 << 23, scalar2=-1.0,
                                        op0=A.subtract, op1=A.mult)
                s1 = s1i[:].bitcast(DT)
                # ---- quantize ----
                if quant == "int8":
                    # f32->int8 output convert == RNE round + saturate to
                    # [-128,127]: exactly clip(round_half_even(x*s1));
                    # verified identical on DVE and ACT.  With alt_quant,
                    # odd tiles swap which engine does quant vs dequant so
                    # each engine sees one big pass per tile.
                    q8 = big.tile([P, tile_f], mybir.dt.int8, tag="q8")
                    qe = dequant_engine if (alt_quant and j % 2) else "vector"
                    de = ("vector" if (alt_quant and j % 2)
                          else dequant_engine)
                    if qe == "scalar":
                        nc.scalar.activation(
                            out=q8[:], in_=t[:],
                            func=mybir.ActivationFunctionType.Copy,
                            bias=0.0, scale=s1)
                    else:
                        nc.vector.tensor_scalar(out=q8[:], in0=t[:],
                                                scalar1=s1, scalar2=None,
                                                op0=A.mult)
                    to = t if out_dtype == "f32" else big.tile(
                        [P, tile_f], DO, tag="to")
                    if de == "scalar":
                        nc.scalar.activation(
                            out=to[:], in_=q8[:],
                            func=mybir.ActivationFunctionType.Copy,
                            bias=0.0, scale=s2)
                    else:
                        getattr(nc, de).tensor_scalar(
                            out=to[:], in0=q8[:], scalar1=s2, scalar2=None,
                            op0=A.mult)
                else:
                    # magic-number RNE path, all exact in f32
                    nc.vector.tensor_scalar(out=t[:], in0=t[:], scalar1=s1,
                                            scalar2=C_MAGIC,
                                            op0=A.mult, op1=A.add)
                    nc.vector.tensor_scalar(out=t[:], in0=t[:],
                                            scalar1=C_MAGIC + 127.0,
                                            scalar2=C_MAGIC - 128.0,
                                            op0=A.min, op1=A.max)
                    to = t if out_dtype == "f32" else big.tile(
                        [P, tile_f], DO, tag="to")
                    nc.vector.tensor_scalar(out=to[:], in0=t[:],
                                            scalar1=-C_MAGIC, scalar2=s2,
                                            op0=A.add, op1=A.mult)
                getattr(nc, store_engs[j % len(store_engs)]).dma_start(
                    out=blk(y, i), in_=to[:])
            nc.sync.dma_start(out=st[rep * n_t:(rep + 1) * n_t],
                              in_=stats[0:1, :])

    nc.compile()
    return nc


def _get_nc(**kw):
    key = tuple(sorted(kw.items()))
    if key not in _CACHE:
        _CACHE[key] = _build(**kw)
    return _CACHE[key]


def _np_out_dtype():
    import ml_dtypes
    return np.dtype(ml_dtypes.bfloat16)


def _get_fn():
    """Jitted 8-core executable, compiled once and reused across calls."""
    if "fn" in _CACHE:
        return _CACHE["fn"]
    import jax
    import jax.numpy as jnp
    from jax.sharding import Mesh, NamedSharding, PartitionSpec
    from jax.experimental.shard_map import shard_map
    from concourse import bass2jax
    from concourse.bass2jax import _bass_exec_p, partition_id_tensor

    bass2jax.install_neuronx_cc_hook()
    nc = _get_nc()
    devices = jax.devices()[:N_CORES]
    mesh = Mesh(np.asarray(devices), ("core",))
    y_aval = jax.core.ShapedArray((NELEM,), jnp.bfloat16)
    st_aval = jax.core.ShapedArray((N_T,), np.float32)

    def _body(xa, ya, sa):
        outs = _bass_exec_p.bind(
            xa, ya, sa, partition_id_tensor(),
            out_avals=(y_aval, st_aval),
            in_names=("x", "y", "st", nc.partition_id_tensor.name),
            out_names=("y", "st"),
            lowering_input_output_aliases=(),
            sim_require_finite=True,
            sim_require_nnan=True,
            nc=nc,
        )
        return outs[0], outs[1]

    fn = jax.jit(shard_map(
        _body, mesh=mesh,
        in_specs=(PartitionSpec("core"),) * 3,
        out_specs=(PartitionSpec("core"), PartitionSpec("core")),
        check_rep=False))
    sharding = NamedSharding(mesh, PartitionSpec("core"))
    # output operand buffers: materialized on device once and reused across
    # calls -- never mutated since the custom call's results are fresh
    yd = jax.jit(lambda: jnp.zeros((N_CORES * NELEM,), jnp.bfloat16),
                 out_shardings=sharding)()
    sd = jax.jit(lambda: jnp.zeros((N_CORES * N_T,), jnp.float32),
                 out_shardings=sharding)()
    yd.block_until_ready()
    sd.block_until_ready()
    _CACHE["fn"] = (fn, sharding, yd, sd)
    return _CACHE["fn"]


def _exponent(v):
    """floor(log2(v)) for positive finite v, exact (frexp)."""
    m, ex = np.frexp(np.float32(v))
    return int(ex) - 1


def kernel(x: np.ndarray) -> np.ndarray:
    import jax

    x = np.ascontiguousarray(np.asarray(x), dtype=np.float32)
    assert x.shape == (B, S, D), x.shape
    fn, sharding, yd, sd = _get_fn()
    xd = jax.device_put(x.reshape(N_CORES * NELEM), sharding)
    out, stats = fn(xd, yd, sd)
    stats = np.asarray(stats)                     # (N_CORES * N_T,)
    y = np.asarray(out).astype(np.float32)        # bf16 -> f32, exact
    gmax = max(float(stats.max()), 1e-10)
    e_ref = min(max(_exponent(gmax), -128), 127)
    # a tile is exact iff the device used the same exponent octave and all
    # scales/outputs were normal fp32/bf16 (guaranteed when e >= -120)
    good = np.array([s > 0 and _exponent(s) == e_ref for s in stats])
    if good.all() and -120 <= e_ref <= 127:
        return y.reshape(B, S, D)
    # rare path: re-quantize the mismatched tiles exactly on the host
    y = y.reshape(N_CORES * N_T, P * TILE_F)
    xr = x.reshape(N_CORES * N_T, P * TILE_F)
    s1 = np.float32(2.0 ** (6 - e_ref))
    s2 = np.float32(2.0 ** (e_ref - 6))
    bad = ~good if -120 <= e_ref <= 127 else np.ones_like(good)
    for j in np.nonzero(bad)[0]:
        i = np.clip(np.rint(xr[j] * s1), -128.0, 127.0).astype(np.float32)
        y[j] = i * s2
    return y.reshape(B, S, D)
